# revision 6
# baseline (speedup 1.0000x reference)
"""Trainium2 Bass kernel for nn_ConformerBlock_50525995270849.

Takes FULL unsharded inputs (as produced by setup_inputs()) and returns the
FULL [B, D, T] fp32 output, running on 8 NeuronCores via run_bass_kernel_spmd.

Sharding: core c = (batch b=c//2, T-half parity p=c%2). Each core computes
FFN1+QKV over full T (K/V need all positions), attention for its extended
640-column local query frame (global cols [384p, 384p+640)), and the conv
module + FFN2 for its own 512 columns (local frame cols [128p, 128p+512)).
GroupNorm(1,C) statistics are globally reduced with the pair core via two
tiny AllReduce collectives. The gated relative-position bias is added to the
score PSUM with per-t-block diag(f) matmuls against diagonal table
expansions (negative-free-step DMA from per-core shifted tables).

Algebra validated against the reference in numpy (rel err 3.2e-3, bf16).
"""
import sys
import os

sys.path.insert(0, "/opt/trn_rl_repo")

import numpy as np
import ml_dtypes

B, T, D, H, DH = 4, 1024, 512, 8, 64
FF = 4 * D
KW = 31
NB, MAXD = 320, 800
PAD = KW // 2
NCORES = 8
P = 128
FRAME = 640          # local query frame width
OWN = 512            # own region width
TABW = 1664          # per-core bias table width

bfnp = ml_dtypes.bfloat16
_CACHE = {}


def bucket1d():
    half, thr = NB // 2, NB // 4
    r = np.arange(-(T - 1), T, dtype=np.int32)
    sign = (r >= 0).astype(np.int32)
    ap = np.abs(r)
    log_ratio = np.log(np.maximum(ap, 1).astype(np.float32) / thr) / np.float32(
        np.log(MAXD / thr))
    log_pos = np.minimum(np.rint(thr + log_ratio * (half - thr)).astype(np.int32),
                         half - 1)
    return np.clip(np.where(ap < thr, ap, log_pos) + sign * half, 0, NB - 1)


# column layout of the packed per-partition bias tensor cbias [128, NCB]
_CB = {}
_ncb = 0
for _name, _n in [("b1a", 16), ("b2a", 4), ("bq", 4), ("bk", 4), ("bo", 4),
                  ("Wg", 8), ("Wb", 8), ("dwb", 4), ("g2", 4), ("b2g", 4),
                  ("bpw2", 4), ("b1b", 16), ("b2b", 4), ("sh", 8), ("dw", 31 * 4)]:
    _CB[_name] = _ncb
    _ncb += _n
NCB = _ncb


def _build_program():
    import concourse.bass as bass
    import concourse.tile as tile
    from concourse import bacc, mybir
    from contextlib import ExitStack

    f32 = mybir.dt.float32
    bf16 = mybir.dt.bfloat16
    AF = mybir.ActivationFunctionType
    OP = mybir.AluOpType
    AX = mybir.AxisListType

    nc = bacc.Bacc("TRN2", target_bir_lowering=False, debug=False,
                   num_devices=NCORES)

    di = lambda n, s, dt: nc.dram_tensor(n, s, dt, kind="ExternalInput")
    xb_d = di("xb", [D, T], bf16)
    w1a_d = di("w1a", [D, FF], bf16)
    w2a_d = di("w2a", [FF, D], bf16)
    wqkvo_d = di("wqkvo", [D, 4 * D], bf16)   # [wq | wk | wv | wo]
    pw1g_d = di("pw1g", [D, 2 * D], bf16)
    pw2t_d = di("pw2t", [D, D], bf16)
    w1b_d = di("w1b", [D, FF], bf16)
    w2b_d = di("w2b", [FF, D], bf16)
    gg_d = di("gg", [D, 16], bf16)
    i128_d = di("i128", [P, P], bf16)
    ones1_d = di("ones1", [1, P], bf16)
    bvrow_d = di("bvrow", [1, D], bf16)
    cbias_d = di("cbias", [P, NCB], f32)
    tab_d = di("tab", [H, TABW], bf16)
    toff_d = di("toff", [1, 1], mybir.dt.uint32)
    own0_d = di("own0", [1, 1], mybir.dt.uint32)
    y_d = nc.dram_tensor("y", [D, OWN], f32, kind="ExternalOutput").ap()

    with tile.TileContext(nc) as tc, ExitStack() as ctx:
        pool = lambda name, bufs, **kw: ctx.enter_context(
            tc.tile_pool(name=name, bufs=bufs, **kw))
        cp = pool("const", 1)
        big = pool("big", 1)
        wpA = pool("wpA", 4)        # [128, 2048] bf16 weight row-tiles
        wpB = pool("wpB", 17)       # [128, 512] bf16 weight row-tiles
        hp = pool("hp", 17)         # FFN hidden tiles [128, 512] bf16
        qrot = pool("qrot", 2)      # q full-T tiles (consumed by qloc copy)
        Pp = pool("Pp", 9)          # attention probs tiles [128, 640] bf16
        dgp = pool("dgp", 6)        # diag tiles [128, 128] bf16
        dbp = pool("dbp", 12)       # bias table blocks [128, 128] bf16
        scp = pool("scp", 2)        # small scratch
        drp = pool("drp", 1, space="DRAM")
        psA = pool("psA", 2, space="PSUM")
        psB = pool("psB", 3, space="PSUM")

        # ---- registers (per-engine) for the two dynamic offsets ----
        tr_v = nc.vector.alloc_register("toff_v")
        nc.vector.reg_load(tr_v, toff_d[0:1, 0:1])
        toff_v = nc.vector.snap(tr_v, donate=True, min_val=0, max_val=384)
        ow_v = nc.vector.alloc_register("own0_v")
        nc.vector.reg_load(ow_v, own0_d[0:1, 0:1])
        own0_v = nc.vector.snap(ow_v, donate=True, min_val=0, max_val=128)
        ow_s = nc.scalar.alloc_register("own0_s")
        nc.scalar.reg_load(ow_s, own0_d[0:1, 0:1])
        own0_s = nc.scalar.snap(ow_s, donate=True, min_val=0, max_val=128)

        # ---- constants ----
        cbias = cp.tile([P, NCB], f32)
        nc.sync.dma_start(cbias[:], cbias_d.ap())
        i128 = cp.tile([P, P], bf16)
        nc.sync.dma_start(i128[:], i128_d.ap())
        ones1 = cp.tile([1, P], bf16)
        nc.sync.dma_start(ones1[:], ones1_d.ap())
        bvrow = cp.tile([1, D], bf16)
        nc.sync.dma_start(bvrow[:], bvrow_d.ap())
        gg_sb = [cp.tile([P, 16], bf16, tag=f"gg{k}", name=f"gg{k}")
                 for k in range(4)]
        for k in range(4):
            nc.sync.dma_start(gg_sb[k][:], gg_d.ap()[k * P:(k + 1) * P, :])

        def cb(name, i):
            return cbias[:, _CB[name] + i:_CB[name] + i + 1]

        # ---- persistent activation buffers ----
        bt = lambda tg, w, dt=bf16: big.tile([P, w], dt, tag=tg, name=tg)
        xb = [bt(f"xb{k}", T) for k in range(4)]
        for k in range(4):
            nc.sync.dma_start(xb[k][:], xb_d.ap()[k * P:(k + 1) * P, :])
        s1f = [bt(f"s1f{k}", T, f32) for k in range(4)]
        s1b = [bt(f"s1b{k}", T) for k in range(4)]
        kkb = [bt(f"kkb{k}", T) for k in range(4)]
        vT = [bt(f"vT{t}", 520) for t in range(8)]
        qloc = [bt(f"qloc{k}", FRAME) for k in range(4)]
        fsb = [bt(f"fsb{t}", 8, f32) for t in range(5)]
        ohat = [bt(f"ohat{k}", FRAME) for k in range(4)]
        s2f = [bt(f"s2f{k}", FRAME, f32) for k in range(4)]
        s2b = [bt(f"s2b{k}", FRAME) for k in range(4)]
        a_sb = [bt(f"a{m}", FRAME) for m in range(4)]
        sg_sb = [bt(f"sg{m}", FRAME) for m in range(4)]
        glu = [bt(f"glu{m}", FRAME + 2 * PAD) for m in range(4)]
        dcb = [bt(f"dcb{m}", FRAME) for m in range(4)]
        slown = [bt(f"slown{m}", OWN) for m in range(4)]
        c2f = [bt(f"c2f{m}", OWN, f32) for m in range(4)]
        c2b = [bt(f"c2b{m}", OWN) for m in range(4)]
        stats = bt("stats", 8, f32)
        stats2 = bt("stats2", 2, f32)
        statsB = bt("statsB", 8, f32)
        stats2B = bt("stats2B", 2, f32)
        gla = bt("gla", 2, f32)     # gn1 allreduced [sum, sq]
        glb = bt("glb", 2, f32)     # gn2 allreduced
        r1b = bt("r1b", 2, f32)     # [r1, -m1*r1]
        r2b = bt("r2b", 2, f32)     # [r2, -m2]
        biasg = bt("biasg", 8, f32)
        sact = bt("sact", 4, f32)
        bact = bt("bact", 4, f32)
        scr2 = bt("scr2", 8, f32)   # scalar scratch columns

        # =========== Phase 1: FFN1 over full T ===========
        w1a_t = []
        for k in range(4):
            w = wpA.tile([P, FF], bf16, tag="wA", name=f"w1a{k}")
            nc.sync.dma_start(w[:], w1a_d.ap()[k * P:(k + 1) * P, :])
            w1a_t.append(w)
        w2a_t = []
        for k in range(16):
            w = wpB.tile([P, D], bf16, tag="wB", name=f"w2a{k}")
            nc.sync.dma_start(w[:], w2a_d.ap()[k * P:(k + 1) * P, :])
            w2a_t.append(w)
        for tch in range(2):
            c0 = tch * 512
            hs = []
            for m in range(16):
                ps = psA.tile([P, 512], f32, tag="psA", name=f"ps1_{tch}_{m}")
                for k in range(4):
                    nc.tensor.matmul(ps[:], w1a_t[k][:, m * P:(m + 1) * P],
                                     xb[k][:, c0:c0 + 512],
                                     start=(k == 0), stop=(k == 3))
                ht = hp.tile([P, 512], bf16, tag="h", name=f"h_{tch}_{m}")
                nc.scalar.activation(ht[:], ps[:], AF.Gelu, bias=cb("b1a", m))
                hs.append(ht)
            for m in range(4):
                ps = psA.tile([P, 512], f32, tag="psA", name=f"ps2_{tch}_{m}")
                for k in range(16):
                    nc.tensor.matmul(ps[:], w2a_t[k][:, m * P:(m + 1) * P],
                                     hs[k][:], start=(k == 0), stop=(k == 15))
                nc.vector.scalar_tensor_tensor(
                    s1f[m][:, c0:c0 + 512], ps[:], cb("b2a", m),
                    xb[m][:, c0:c0 + 512], OP.add, OP.add)
                nc.vector.tensor_copy(s1b[m][:, c0:c0 + 512],
                                      s1f[m][:, c0:c0 + 512])

        # =========== Phase 2: QKV ===========
        wq_t, wk_t, wv_t, wo_t = [], [], [], []
        for k in range(4):
            w = wpA.tile([P, 4 * D], bf16, tag="wA", name=f"wqkvo{k}")
            nc.sync.dma_start(w[:], wqkvo_d.ap()[k * P:(k + 1) * P, :])
            wq_t.append(w[:, 0:D])
            wk_t.append(w[:, D:2 * D])
            wv_t.append(w[:, 2 * D:3 * D])
            wo_t.append(w[:, 3 * D:4 * D])
        # q (full T, rotating) -> qloc (static local frame)
        qfull = []
        for m in range(4):
            qf = qrot.tile([P, T], bf16, tag="qf", name=f"qf{m}")
            for tch in range(2):
                c0 = tch * 512
                ps = psA.tile([P, 512], f32, tag="psA", name=f"psq{m}{tch}")
                for k in range(4):
                    nc.tensor.matmul(ps[:], wq_t[k][:, m * P:(m + 1) * P],
                                     s1b[k][:, c0:c0 + 512],
                                     start=(k == 0), stop=(k == 3))
                nc.scalar.activation(qf[:, c0:c0 + 512], ps[:], AF.Identity,
                                     bias=cb("bq", m))
            nc.vector.tensor_copy(qloc[m][:],
                                  qf[:, bass.ds(toff_v, FRAME)])
            qfull.append(qf)
        # k (full T, persistent)
        for m in range(4):
            for tch in range(2):
                c0 = tch * 512
                ps = psA.tile([P, 512], f32, tag="psA", name=f"psk{m}{tch}")
                for k in range(4):
                    nc.tensor.matmul(ps[:], wk_t[k][:, m * P:(m + 1) * P],
                                     s1b[k][:, c0:c0 + 512],
                                     start=(k == 0), stop=(k == 3))
                nc.scalar.activation(kkb[m][:, c0:c0 + 512], ps[:], AF.Identity,
                                     bias=cb("bk", m))
        # vT: [t, dv] with ones columns (65-block layout)
        for t in range(8):
            ps = psA.tile([P, 512], f32, tag="psA", name=f"psv{t}")
            for k in range(4):
                nc.tensor.matmul(ps[:], s1b[k][:, t * P:(t + 1) * P],
                                 wv_t[k][:], start=(k == 0), stop=False)
            nc.tensor.matmul(ps[:], ones1[:, 0:P], bvrow[:],
                             start=False, stop=True)
            src3 = ps[:].rearrange("p (h c) -> p h c", c=64)
            dst3 = vT[t][:, 0:520].rearrange("p (h c) -> p h c", c=65)[:, :, 0:64]
            nc.scalar.activation(dst3, src3, AF.Copy)
            onescol = vT[t][:, 0:520].rearrange("p (h c) -> p h c", c=65)[:, :, 64:65]
            nc.gpsimd.memset(onescol, 1.0)

        # =========== Phase 3: gates -> f ===========
        for tt in range(5):
            ps = psA.tile([P, 512], f32, tag="psA", name=f"psg{tt}")
            for k in range(4):
                nc.tensor.matmul(ps[:, 0:16], qloc[k][:, tt * P:(tt + 1) * P],
                                 gg_sb[k][:], start=(k == 0), stop=(k == 3))
            sgt = scp.tile([P, 16], f32, tag="sgt", name=f"sgt{tt}")
            nc.scalar.activation(sgt[:], ps[:, 0:16], AF.Sigmoid)
            gm = scp.tile([P, 8], f32, tag="gm", name=f"gm{tt}")
            nc.vector.tensor_tensor(gm[:], sgt[:, 0:8], sgt[:, 8:16], OP.mult)
            gd = scp.tile([P, 8], f32, tag="gd", name=f"gd{tt}")
            nc.vector.tensor_tensor(gd[:], sgt[:, 8:16], gm[:], OP.subtract)
            gs = scp.tile([P, 8], f32, tag="gs", name=f"gs{tt}")
            nc.vector.tensor_tensor(gs[:], gd[:], cbias[:, _CB["sh"]:_CB["sh"] + 8],
                                    OP.mult)
            nc.vector.scalar_tensor_tensor(fsb[tt][:], gs[:], 1.0, sgt[:, 0:8],
                                           OP.add, OP.add)

        # =========== Phase 4: attention per head ===========
        for h in range(H):
            kt = h // 2
            pb = 64 * (h % 2)
            dgs = []
            for j in range(5):
                dg = dgp.tile([P, P], bf16, tag="dg", name=f"dg{h}_{j}")
                nc.vector.tensor_scalar(dg[:], i128[:], fsb[j][:, h:h + 1], None,
                                        OP.mult)
                dgs.append(dg)
            psv = psB.tile([P, FRAME], f32, tag="psB", name=f"psav{h}")
            Pts = []
            for st in range(8):
                ps = psB.tile([P, FRAME], f32, tag="psB", name=f"pssc{h}_{st}")
                nc.tensor.matmul(ps[:, 0:512],
                                 kkb[kt][pb:pb + 64, st * P:(st + 1) * P],
                                 qloc[kt][pb:pb + 64, 0:512],
                                 start=True, stop=False)
                nc.tensor.matmul(ps[:, 512:FRAME],
                                 kkb[kt][pb:pb + 64, st * P:(st + 1) * P],
                                 qloc[kt][pb:pb + 64, 512:FRAME],
                                 start=True, stop=False)
                for j in range(5):
                    C0 = 1023 + 128 * j - 128 * st
                    db = dbp.tile([P, P], bf16, tag="db", name=f"db{h}_{st}_{j}")
                    nc.sync.dma_start(db[:], bass.AP(tab_d, h * TABW + C0,
                                                     [[1, P], [-1, P]]))
                    nc.tensor.matmul(ps[:, j * P:(j + 1) * P], db[:], dgs[j][:],
                                     start=False, stop=(j == 4))
                Pt = Pp.tile([P, FRAME], bf16, tag="P", name=f"P{h}_{st}")
                nc.scalar.activation(Pt[:], ps[:], AF.Exp)
                Pts.append(Pt)
            for st in range(8):
                nc.tensor.matmul(psv[0:65, 0:512], vT[st][:, 65 * h:65 * h + 65],
                                 Pts[st][:, 0:512],
                                 start=(st == 0), stop=(st == 7))
                nc.tensor.matmul(psv[0:65, 512:FRAME],
                                 vT[st][:, 65 * h:65 * h + 65],
                                 Pts[st][:, 512:FRAME],
                                 start=(st == 0), stop=(st == 7))
            rc = scp.tile([1, FRAME], bf16, tag="rc", name=f"rc{h}")
            with nc.allow_low_precision(reason="softmax recip colsum, bf16 ok"):
                nc.vector.reciprocal(rc[:], psv[64:65, 0:FRAME])
            pb1 = psA.tile([P, 512], f32, tag="psA", name=f"psbc{h}a")
            nc.tensor.matmul(pb1[0:64, 0:512], ones1[:, 0:64], rc[:, 0:512],
                             start=True, stop=True)
            pb2 = psA.tile([P, 512], f32, tag="psA", name=f"psbc{h}b")
            nc.tensor.matmul(pb2[0:64, 0:128], ones1[:, 0:64], rc[:, 512:FRAME],
                             start=True, stop=True)
            rb = scp.tile([64, FRAME], bf16, tag="rb", name=f"rb{h}")
            nc.scalar.activation(rb[:, 0:512], pb1[0:64, 0:512], AF.Copy)
            nc.scalar.activation(rb[:, 512:FRAME], pb2[0:64, 0:128], AF.Copy)
            nc.vector.tensor_tensor(ohat[kt][pb:pb + 64, :], psv[0:64, 0:FRAME],
                                    rb[:], OP.mult)

        # =========== Phase 5: out-proj + residual -> s2 ===========
        for m in range(4):
            ps = psB.tile([P, FRAME], f32, tag="psB", name=f"pso{m}")
            for k in range(4):
                nc.tensor.matmul(ps[:, 0:512], wo_t[k][:, m * P:(m + 1) * P],
                                 ohat[k][:, 0:512], start=(k == 0), stop=(k == 3))
                nc.tensor.matmul(ps[:, 512:FRAME], wo_t[k][:, m * P:(m + 1) * P],
                                 ohat[k][:, 512:FRAME],
                                 start=(k == 0), stop=(k == 3))
            nc.vector.scalar_tensor_tensor(
                s2f[m][:], ps[:], cb("bo", m),
                s1f[m][:, bass.ds(toff_v, FRAME)], OP.add, OP.add)
            nc.vector.tensor_copy(s2b[m][:], s2f[m][:])

        # =========== Phase 6: gn1 stats + pair AllReduce ===========
        nc.gpsimd.memset(stats[:], 0.0)
        sqscr = [scp.tile([P, OWN], bf16, tag="sqscr", name=f"sqs{m}")
                 for m in range(4)]
        for m in range(4):
            nc.vector.tensor_reduce(stats[:, m:m + 1],
                                    s2f[m][:, bass.ds(own0_v, OWN)],
                                    AX.X, OP.add)
            nc.scalar.activation(sqscr[m][:], s2f[m][:, bass.ds(own0_s, OWN)],
                                 AF.Square, accum_out=stats[:, 4 + m:5 + m])
        nc.vector.tensor_reduce(stats2[:, 0:1], stats[:, 0:4], AX.X, OP.add)
        nc.vector.tensor_reduce(stats2[:, 1:2], stats[:, 4:8], AX.X, OP.add)
        cc1i = drp.tile([P, 2], f32, tag="cc1i", name="cc1i")
        cc1o = drp.tile([P, 2], f32, tag="cc1o", name="cc1o")
        nc.sync.dma_start(cc1i[:], stats2[:])
        nc.gpsimd.collective_compute(
            "AllReduce", OP.add,
            replica_groups=[[0, 1], [2, 3], [4, 5], [6, 7]],
            ins=[cc1i[:]], outs=[cc1o[:]])
        nc.sync.dma_start(gla[:], cc1o[:])
        import concourse.bass_isa as bass_isa
        nc.gpsimd.partition_all_reduce(gla[:], gla[:], P, bass_isa.ReduceOp.add)
        # r1 = 1/sqrt(var+eps); r1b = [r1, -m1*r1]
        n_inv = 1.0 / float(D * T)
        nc.vector.tensor_scalar(scr2[:, 0:1], gla[:, 0:1], n_inv, None, OP.mult)
        nc.vector.tensor_scalar(scr2[:, 1:2], gla[:, 1:2], n_inv, None, OP.mult)
        nc.vector.tensor_tensor(scr2[:, 2:3], scr2[:, 0:1], scr2[:, 0:1], OP.mult)
        nc.vector.tensor_tensor(scr2[:, 3:4], scr2[:, 1:2], scr2[:, 2:3],
                                OP.subtract)
        nc.vector.tensor_scalar(scr2[:, 3:4], scr2[:, 3:4], 1e-5, None, OP.add)
        nc.scalar.activation(scr2[:, 4:5], scr2[:, 3:4], AF.Sqrt)
        nc.vector.reciprocal(r1b[:, 0:1], scr2[:, 4:5])
        nc.vector.tensor_tensor(scr2[:, 5:6], scr2[:, 0:1], r1b[:, 0:1], OP.mult)
        nc.vector.tensor_scalar(r1b[:, 1:2], scr2[:, 5:6], -1.0, None, OP.mult)
        for m in range(8):
            nc.vector.scalar_tensor_tensor(biasg[:, m:m + 1], cb("Wg", m),
                                           r1b[:, 1:2], cb("Wb", m),
                                           OP.mult, OP.add)

        # =========== Phase 7: pw1 + GLU ===========
        pw1_t = []
        for k in range(4):
            w = wpA.tile([P, 2 * D], bf16, tag="wA", name=f"pw1g{k}")
            nc.sync.dma_start(w[:], pw1g_d.ap()[k * P:(k + 1) * P, :])
            pw1_t.append(w)
        for m in range(8):
            ps = psB.tile([P, FRAME], f32, tag="psB", name=f"psp1{m}")
            for k in range(4):
                nc.tensor.matmul(ps[:, 0:512], pw1_t[k][:, m * P:(m + 1) * P],
                                 s2b[k][:, 0:512], start=(k == 0), stop=(k == 3))
                nc.tensor.matmul(ps[:, 512:FRAME], pw1_t[k][:, m * P:(m + 1) * P],
                                 s2b[k][:, 512:FRAME],
                                 start=(k == 0), stop=(k == 3))
            if m < 4:
                nc.vector.tensor_scalar(a_sb[m][:], ps[:], r1b[:, 0:1],
                                        biasg[:, m:m + 1], OP.mult, OP.add)
            else:
                nc.scalar.activation(sg_sb[m - 4][:], ps[:], AF.Sigmoid,
                                     bias=biasg[:, m:m + 1], scale=r1b[:, 0:1])
        for m in range(4):
            nc.gpsimd.memset(glu[m][:, 0:PAD], 0.0)
            nc.gpsimd.memset(glu[m][:, PAD + FRAME:FRAME + 2 * PAD], 0.0)
            nc.vector.tensor_tensor(glu[m][:, PAD:PAD + FRAME], a_sb[m][:],
                                    sg_sb[m][:], OP.mult)

        # =========== Phase 8: depthwise conv (diag matmuls) ===========
        for m in range(4):
            ps = psB.tile([P, FRAME], f32, tag="psB", name=f"psdc{m}")
            for k in range(KW):
                dg = dgp.tile([P, P], bf16, tag="dg", name=f"dwg{m}_{k}")
                nc.vector.tensor_scalar(dg[:], i128[:], cb("dw", m * KW + k),
                                        None, OP.mult)
                nc.tensor.matmul(ps[:, 0:512], dg[:], glu[m][:, k:k + 512],
                                 start=(k == 0), stop=False)
                nc.tensor.matmul(ps[:, 512:FRAME], dg[:],
                                 glu[m][:, k + 512:k + FRAME],
                                 start=(k == 0), stop=(k == KW - 1))
            nc.scalar.activation(dcb[m][:], ps[:], AF.Copy)

        # =========== Phase 9: gn2 stats + pair AllReduce ===========
        nc.gpsimd.memset(statsB[:], 0.0)
        sqscr2 = [scp.tile([P, OWN], bf16, tag="sqscr", name=f"sqs2{m}")
                  for m in range(4)]
        sc_t = scp.tile([P, 4], f32, tag="sct", name="sct")
        for m in range(4):
            nc.vector.tensor_reduce(sc_t[:, m:m + 1],
                                    dcb[m][:, bass.ds(own0_v, OWN)],
                                    AX.X, OP.add)
            nc.scalar.activation(sqscr2[m][:], dcb[m][:, bass.ds(own0_s, OWN)],
                                 AF.Square, accum_out=statsB[:, 4 + m:5 + m])
        for m in range(4):
            # sum_adj = sc + 512*dwb ; sq_adj = sq + 2*dwb*sc + 512*dwb^2
            nc.vector.scalar_tensor_tensor(statsB[:, m:m + 1], cb("dwb", m),
                                           512.0, sc_t[:, m:m + 1],
                                           OP.mult, OP.add)
            nc.vector.tensor_tensor(scr2[:, 6:7], cb("dwb", m), sc_t[:, m:m + 1],
                                    OP.mult)
            nc.vector.scalar_tensor_tensor(scr2[:, 7:8], scr2[:, 6:7], 2.0,
                                           statsB[:, 4 + m:5 + m],
                                           OP.mult, OP.add)
            nc.vector.tensor_tensor(scr2[:, 6:7], cb("dwb", m), cb("dwb", m),
                                    OP.mult)
            nc.vector.scalar_tensor_tensor(statsB[:, 4 + m:5 + m], scr2[:, 6:7],
                                           512.0, scr2[:, 7:8], OP.mult, OP.add)
        nc.vector.tensor_reduce(stats2B[:, 0:1], statsB[:, 0:4], AX.X, OP.add)
        nc.vector.tensor_reduce(stats2B[:, 1:2], statsB[:, 4:8], AX.X, OP.add)
        cc2i = drp.tile([P, 2], f32, tag="cc2i", name="cc2i")
        cc2o = drp.tile([P, 2], f32, tag="cc2o", name="cc2o")
        nc.sync.dma_start(cc2i[:], stats2B[:])
        nc.gpsimd.collective_compute(
            "AllReduce", OP.add,
            replica_groups=[[0, 1], [2, 3], [4, 5], [6, 7]],
            ins=[cc2i[:]], outs=[cc2o[:]])
        nc.sync.dma_start(glb[:], cc2o[:])
        nc.gpsimd.partition_all_reduce(glb[:], glb[:], P, bass_isa.ReduceOp.add)
        nc.vector.tensor_scalar(scr2[:, 0:1], glb[:, 0:1], n_inv, None, OP.mult)
        nc.vector.tensor_scalar(scr2[:, 1:2], glb[:, 1:2], n_inv, None, OP.mult)
        nc.vector.tensor_tensor(scr2[:, 2:3], scr2[:, 0:1], scr2[:, 0:1], OP.mult)
        nc.vector.tensor_tensor(scr2[:, 3:4], scr2[:, 1:2], scr2[:, 2:3],
                                OP.subtract)
        nc.vector.tensor_scalar(scr2[:, 3:4], scr2[:, 3:4], 1e-5, None, OP.add)
        nc.scalar.activation(scr2[:, 4:5], scr2[:, 3:4], AF.Sqrt)
        nc.vector.reciprocal(r2b[:, 0:1], scr2[:, 4:5])
        nc.vector.tensor_scalar(r2b[:, 1:2], scr2[:, 0:1], -1.0, None, OP.mult)
        for m in range(4):
            # sact = r2*g2 ; bact = sact*(dwb - m2) + b2g
            nc.vector.tensor_tensor(sact[:, m:m + 1], cb("g2", m), r2b[:, 0:1],
                                    OP.mult)
            nc.vector.tensor_tensor(scr2[:, 6:7], cb("dwb", m), r2b[:, 1:2],
                                    OP.add)  # dwb + (-m2)
            nc.vector.tensor_tensor(scr2[:, 7:8], scr2[:, 6:7], sact[:, m:m + 1],
                                    OP.mult)
            nc.vector.tensor_tensor(bact[:, m:m + 1], scr2[:, 7:8], cb("b2g", m),
                                    OP.add)
        # silu over own region only
        for m in range(4):
            nc.scalar.activation(slown[m][:], dcb[m][:, bass.ds(own0_s, OWN)],
                                 AF.Silu, bias=bact[:, m:m + 1],
                                 scale=sact[:, m:m + 1])

        # =========== Phase 10: pw2 + residual -> c2 ===========
        pw2_t = []
        for k in range(4):
            w = wpB.tile([P, D], bf16, tag="wB", name=f"pw2t{k}")
            nc.sync.dma_start(w[:], pw2t_d.ap()[k * P:(k + 1) * P, :])
            pw2_t.append(w)
        for m in range(4):
            ps = psA.tile([P, 512], f32, tag="psA", name=f"psp2{m}")
            for k in range(4):
                nc.tensor.matmul(ps[:], pw2_t[k][:, m * P:(m + 1) * P],
                                 slown[k][:], start=(k == 0), stop=(k == 3))
            nc.vector.scalar_tensor_tensor(
                c2f[m][:], ps[:], cb("bpw2", m),
                s2f[m][:, bass.ds(own0_v, OWN)], OP.add, OP.add)
            nc.vector.tensor_copy(c2b[m][:], c2f[m][:])

        # =========== Phase 11: FFN2 over own region ===========
        w1b_t = []
        for k in range(4):
            w = wpA.tile([P, FF], bf16, tag="wA", name=f"w1b{k}")
            nc.sync.dma_start(w[:], w1b_d.ap()[k * P:(k + 1) * P, :])
            w1b_t.append(w)
        w2b_t = []
        for k in range(16):
            w = wpB.tile([P, D], bf16, tag="wB", name=f"w2b{k}")
            nc.sync.dma_start(w[:], w2b_d.ap()[k * P:(k + 1) * P, :])
            w2b_t.append(w)
        h2s = []
        for m in range(16):
            ps = psA.tile([P, 512], f32, tag="psA", name=f"psf2{m}")
            for k in range(4):
                nc.tensor.matmul(ps[:], w1b_t[k][:, m * P:(m + 1) * P],
                                 c2b[k][:], start=(k == 0), stop=(k == 3))
            ht = hp.tile([P, 512], bf16, tag="h", name=f"h2_{m}")
            nc.scalar.activation(ht[:], ps[:], AF.Gelu, bias=cb("b1b", m))
            h2s.append(ht)
        for m in range(4):
            ps = psA.tile([P, 512], f32, tag="psA", name=f"psy{m}")
            for k in range(16):
                nc.tensor.matmul(ps[:], w2b_t[k][:, m * P:(m + 1) * P],
                                 h2s[k][:], start=(k == 0), stop=(k == 15))
            ysb = scp.tile([P, OWN], f32, tag="ysb", name=f"y{m}")
            nc.vector.scalar_tensor_tensor(ysb[:], ps[:], cb("b2b", m),
                                           c2f[m][:], OP.add, OP.add)
            nc.sync.dma_start(y_d[m * P:(m + 1) * P, :], ysb[:])

    nc.compile()
    return nc


def _host_prep(inputs):
    inp = {k: np.asarray(v) for k, v in inputs.items()}
    f32 = np.float32
    g1d = inp["rel_embed"][bucket1d(), :].astype(f32)   # [2047, H]

    tb = lambda a: np.ascontiguousarray(a, dtype=f32).astype(bfnp)
    shared = {
        "w1a": tb(inp["ff1_w1"]),
        "w2a": tb(inp["ff1_w2"] * 0.5),
        "wqkvo": tb(np.concatenate([inp["qkv_w"][:, :D] / 8.0,
                                    inp["qkv_w"][:, D:2 * D],
                                    inp["qkv_w"][:, 2 * D:],
                                    inp["out_w"]], axis=1)),
        "pw1g": tb(inp["pw1_w"].T * inp["gn1_g"][:, None]),
        "pw2t": tb(inp["pw2_w"].T),
        "w1b": tb(inp["ff2_w1"]),
        "w2b": tb(inp["ff2_w2"] * 0.5),
        "i128": np.eye(P, dtype=f32).astype(bfnp),
        "ones1": np.ones((1, P), f32).astype(bfnp),
        "bvrow": tb(inp["qkv_b"][2 * D:][None, :]),
    }
    gg = np.zeros((D, 16), f32)
    for h in range(H):
        gg[64 * h:64 * h + 64, h] = 8.0 * inp["gate_u"][h]
        gg[64 * h:64 * h + 64, 8 + h] = 8.0 * inp["gate_w"][h]
    shared["gg"] = gg.astype(bfnp)

    cbias = np.zeros((P, NCB), f32)

    def put(name, vec, n):
        v = np.asarray(vec, f32).reshape(n, P).T          # [128, n]
        cbias[:, _CB[name]:_CB[name] + n] = v

    put("b1a", inp["ff1_b1"], 16)
    put("b2a", inp["ff1_b2"] * 0.5, 4)
    put("bq", inp["qkv_b"][:D] / 8.0, 4)
    put("bk", inp["qkv_b"][D:2 * D], 4)
    put("bo", inp["out_b"], 4)
    pw1T = inp["pw1_w"].T * inp["gn1_g"][:, None]
    put("Wg", pw1T.sum(axis=0), 8)
    put("Wb", inp["pw1_w"] @ inp["gn1_b"] + inp["pw1_b"], 8)
    put("dwb", inp["dw_b"], 4)
    put("g2", inp["gn2_g"], 4)
    put("b2g", inp["gn2_b"], 4)
    put("bpw2", inp["pw2_b"], 4)
    put("b1b", inp["ff2_b1"], 16)
    put("b2b", inp["ff2_b2"] * 0.5, 4)
    cbias[:, _CB["sh"]:_CB["sh"] + 8] = np.asarray(inp["scale_h"], f32)[None, :]
    dw = np.asarray(inp["dw_w"][:, 0, :], f32)            # [D, KW]
    for m in range(4):
        cbias[:, _CB["dw"] + m * KW:_CB["dw"] + (m + 1) * KW] = \
            dw[m * P:(m + 1) * P, :]
    shared["cbias"] = cbias

    in_maps = []
    for c in range(NCORES):
        b, p = c // 2, c % 2
        tab = np.zeros((H, TABW), f32)
        j = np.arange(TABW)
        idx = 2046 - 384 * p - j
        valid = (idx >= 0) & (idx < 2 * T - 1)
        tab[:, valid] = g1d[idx[valid]].T
        m = dict(shared)
        m["xb"] = np.ascontiguousarray(inp["x"][b], dtype=f32).astype(bfnp)
        m["tab"] = tab.astype(bfnp)
        m["toff"] = np.array([[384 * p]], np.uint32)
        m["own0"] = np.array([[128 * p]], np.uint32)
        in_maps.append(m)
    return in_maps


def get_program():
    if "nc" not in _CACHE:
        _CACHE["nc"] = _build_program()
    return _CACHE["nc"]


def run_cores(inputs, trace=False, **kw):
    from concourse import bass_utils
    nc = get_program()
    in_maps = _host_prep(inputs)
    return bass_utils.run_bass_kernel_spmd(
        nc, in_maps, core_ids=list(range(NCORES)), trace=trace, **kw)


def kernel(**inputs):
    res = run_cores(inputs, trace=False)
    out = np.zeros((B, D, T), np.float32)
    for c in range(NCORES):
        b, p = c // 2, c % 2
        out[b][:, 512 * p:512 * p + 512] = res.results[c]["y"]
    return out


if __name__ == "__main__":
    get_program()
    print("BUILD+COMPILE OK")
